# revision 14
# baseline (speedup 1.0000x reference)
"""Preisach hysteresis (nn_BaseHysteresis) Bass kernel for 8 TRN2 cores.

Math: with shat = (s+1)/2 the per-relay update is affine, shat' = g*shat + c:
    rising  (h > h_prev): g = sigmoid(100*(alpha-h)), c = 1-g
    falling (h < h_prev): g = sigmoid(100*(h-beta)),  c = 0
    equal              : g = 1, c = 0
Since c = mu*(1-g) exactly (mu = rising indicator), the substitution
    w_t = shat_t - mu_t ,  d_t = mu_{t-1} - mu_t   (mu_0 := 0)
turns the recurrence into  w_t = g_t * (w_{t-1} + d_t)  -- no c at all.
The per-step reduction Sum_p dens_p*shat_p,t = Sum_p dens_p*w_p,t
+ mu_t * Sum_p dens_p is fixed up on the host.

Per core (2560 relays = 20 blocks of 128):
 - PE builds arg_g = wg^T @ xg as f32r matmuls into PSUM (half-block
   granularity, double-buffered),
 - ScalarE applies sigmoid PSUM -> G[b%2] (f32, two 1024 halves),
 - DVE runs one tensor_tensor_scan per block (d broadcast tile + G),
   back-to-back -- the 20 scans * ~4.46us are the critical path,
 - PE reduces dens^T @ W into a [1,2048] PSUM accumulator (f32r, lag 2).
GpSimd only memsets the warmup tile: concurrent GpSimd tensor ops halve
DVE scan throughput (measured), so it must stay idle during scans.
Host sums the 8 partial reductions, adds mu*dens_sum, applies the affine.
"""

import os
from contextlib import ExitStack

import ml_dtypes
import numpy as np

import concourse.bass as bass
import concourse.mybir as mybir
from concourse.bass_utils import run_bass_kernel_spmd

F32 = mybir.dt.float32
F32R = mybir.dt.float32r
BF16 = mybir.dt.bfloat16

L = 2048            # field sequence length
P = 128             # SBUF partitions
CHUNK = 512         # PSUM bank free size (f32)
HALF = 1024
NBLK = 20           # relay blocks per core
RCORE = NBLK * P    # relays per core (2560)
NCORES = 8
CAP = RCORE * NCORES  # padded mesh size 20480
M = 20100
BIG = 10000.0
NS = 4              # W-tile ring depth (>= LAG+2, LAG=2)

_last_results = None  # BassKernelResults of the most recent run (for test.py)


def _scan_end(b):
    """s_dve value after block b's scan completes. Block 0 is split
    512/512/1024 (3 instructions), block 19 is split 1536/512 (2)."""
    if b <= 0:
        return 3
    if b >= NBLK - 1:
        return NBLK + 3
    return b + 3


def build_program() -> bass.Bass:
    nc = bass.Bass("TRN2", target_bir_lowering=False)

    xgwg_d = nc.dram_tensor("xgwg", [3, L + RCORE], F32R,
                            kind="ExternalInput")
    s0h_d = nc.dram_tensor("s0h", [P, NBLK], F32, kind="ExternalInput")
    dbc_d = nc.dram_tensor("dbc", [P, L], F32, kind="ExternalInput")
    dens_d = nc.dram_tensor("dens", [P, NBLK], BF16, kind="ExternalInput")
    out_d = nc.dram_tensor("partial", [1, L], F32, kind="ExternalOutput")

    sig = mybir.ActivationFunctionType.Sigmoid
    mult = mybir.AluOpType.mult
    add = mybir.AluOpType.add

    with ExitStack() as ctx:
        xgwg_sb = ctx.enter_context(
            nc.sbuf_tensor([128, L + RCORE], F32R))
        s0h_sb = ctx.enter_context(nc.sbuf_tensor([P, NBLK], F32))
        dbc_sb = ctx.enter_context(nc.sbuf_tensor([P, L], F32))
        dens_sb = ctx.enter_context(nc.sbuf_tensor([P, NBLK], BF16))
        warm = ctx.enter_context(nc.sbuf_tensor([3, CHUNK], BF16))
        scratch = ctx.enter_context(nc.sbuf_tensor([1, 32], F32))
        G = [ctx.enter_context(nc.sbuf_tensor(f"g{i}", [P, L], F32))
             for i in range(2)]
        W = [ctx.enter_context(nc.sbuf_tensor(f"w{i}", [P, L], BF16))
             for i in range(NS)]
        out_sb = ctx.enter_context(nc.sbuf_tensor([1, L], F32))

        PH = [ctx.enter_context(nc.psum_tensor(f"ph{i}", [P, HALF], F32))
              for i in range(2)]
        acc = ctx.enter_context(nc.psum_tensor([1, L], F32))

        s_dma = ctx.enter_context(nc.semaphore("s_dma"))
        s_dm2 = ctx.enter_context(nc.semaphore("s_dm2"))
        s_dmb = ctx.enter_context(nc.semaphore("s_dmb"))
        s_dmc = ctx.enter_context(nc.semaphore("s_dmc"))
        s_warm = ctx.enter_context(nc.semaphore("s_warm"))
        s_arg = ctx.enter_context(nc.semaphore("s_arg"))
        s_red = ctx.enter_context(nc.semaphore("s_red"))
        s_act = ctx.enter_context(nc.semaphore("s_act"))
        s_dve = ctx.enter_context(nc.semaphore("s_dve"))
        block = ctx.enter_context(nc.Block())

        # act completion counts: block0 = 3 acts (512/512/1024),
        # halves x>=2 are act number x+1 -> count x+2; copies follow.
        def act_end(x):
            return (2, 3)[x] if x < 2 else x + 2

        @block.sync
        def _(sync):
            # xg/wg replicated at partition bases 0/32/64 so the PE's stream
            # reads don't concentrate on partitions 0-2 (which would stall
            # the partition-lockstep DVE scans). Copy 0 lands first and
            # gates blocks 0-1; dbc/s0h are issued from the scalar queue in
            # parallel to halve the serialized dma_start issue latency.
            sync.dma_start(xgwg_sb[0:3, :], xgwg_d[:, :]).then_inc(s_dma, 16)
            for c in (1, 2):
                p0 = 32 * c
                sync.dma_start(xgwg_sb[p0:p0 + 3, :], xgwg_d[:, :]
                               ).then_inc(s_dm2, 16)
            sync.dma_start(dens_sb[:, :], dens_d[:, :]).then_inc(s_dmc, 16)
            n_acts = 3 + 2 * (NBLK - 1)
            sync.wait_ge(s_act, n_acts + 1)
            sync.dma_start(out_d[:, 0:HALF], out_sb[:, 0:HALF]
                           ).then_inc(s_dma, 16)
            sync.wait_ge(s_act, n_acts + 2)
            sync.dma_start(out_d[:, HALF:L], out_sb[:, HALF:L]
                           ).then_inc(s_dma, 16)

        @block.gpsimd
        def _(gpsimd):
            gpsimd.memset(warm[:, :], 0.0).then_inc(s_warm, 1)

        @block.tensor
        def _(tensor):
            # brief p-state warmup while the DMAs land
            tensor.wait_ge(s_warm, 1)
            for _ in range(2):
                tensor.matmul(PH[0][:, 0:CHUNK], warm[:, 0:P], warm[:, :],
                              start=True, stop=True, skip_group_check=True)
            tensor.wait_ge(s_dma, 16)   # xg+wg copy 0 loaded
            dens_gate = False

            def emit_dens(j):
                nonlocal dens_gate
                if not dens_gate:
                    tensor.wait_ge(s_dmc, 16)
                    dens_gate = True
                dj = dens_sb[:, j:j + 1]
                wj = W[j % NS]
                if j == NBLK - 1:
                    gates = [NBLK + 2, None, None, NBLK + 3]
                else:
                    gates = [_scan_end(j), None, None, None]
                for k in range(4):
                    sl = slice(k * CHUNK, (k + 1) * CHUNK)
                    if gates[k] is not None:
                        tensor.wait_ge(s_dve, gates[k])
                    mm = tensor.matmul(acc[0:1, sl], dj, wj[:, sl],
                                       start=(j == 0), stop=(j == NBLK - 1),
                                       skip_group_check=True)
                    if k == 3:
                        mm.then_inc(s_red, 1)
                    elif k == 1 and j == NBLK - 1:
                        mm.then_inc(s_red, 1)

            for b in range(NBLK):
                if b == 2:
                    tensor.wait_ge(s_dm2, 2 * 16)   # replicas 1,2 loaded
                for h in range(2):
                    x = 2 * b + h
                    if x >= 2:
                        tensor.wait_ge(s_act, act_end(x - 2))  # PH free
                    lo = h * HALF
                    for j in range(2):
                        # blocks 0-1 only have replica 0 available yet
                        p0 = 0 if b < 2 else 32 * ((2 * h + j) % 3)
                        tensor.matmul(
                            PH[x % 2][:, j * CHUNK:(j + 1) * CHUNK],
                            xgwg_sb[p0:p0 + 3,
                                    L + b * P:L + (b + 1) * P],
                            xgwg_sb[p0:p0 + 3,
                                    lo + j * CHUNK:lo + (j + 1) * CHUNK],
                            start=True, stop=True, skip_group_check=True
                        ).then_inc(s_arg, 1)
                if b >= 2:
                    emit_dens(b - 2)
            emit_dens(NBLK - 2)
            emit_dens(NBLK - 1)

        @block.scalar
        def _(scalar):
            # dbc/s0h loads issue here, in parallel with the sync queue's
            scalar.dma_start(dbc_sb[:, :], dbc_d[:, :]).then_inc(s_dmb, 16)
            scalar.dma_start(s0h_sb[:, :], s0h_d[:, :]).then_inc(s_dmb, 16)
            # sigmoid act-table preload off the critical path
            scalar.wait_ge(s_warm, 1)
            scalar.activation(scratch[:, :], warm[0:1, 0:32], sig)
            # block 0: two 512-wide acts as soon as each arg matmul lands,
            # then one 1024 act for the second half
            for q in range(2):
                scalar.wait_ge(s_arg, q + 1)
                qs = slice(q * CHUNK, (q + 1) * CHUNK)
                scalar.activation(G[0][:, qs], PH[0][:, qs], sig
                                  ).then_inc(s_act, 1)
            scalar.wait_ge(s_arg, 4)
            scalar.activation(G[0][:, HALF:L], PH[1][:, :], sig
                              ).then_inc(s_act, 1)
            for b in range(1, NBLK):
                for h in range(2):
                    x = 2 * b + h
                    scalar.wait_ge(s_arg, 2 * x + 2)
                    if b >= 2:
                        scalar.wait_ge(s_dve, _scan_end(b - 2))  # G free
                    hsl = slice(h * HALF, (h + 1) * HALF)
                    scalar.activation(G[b % 2][:, hsl], PH[x % 2][:, :], sig
                                      ).then_inc(s_act, 1)
            scalar.wait_ge(s_red, NBLK)      # dens(19) chunks 0-1 done
            scalar.copy(out_sb[:, 0:HALF], acc[0:1, 0:HALF]).then_inc(s_act, 1)
            scalar.wait_ge(s_red, NBLK + 1)  # dens(19) chunk 3 done
            scalar.copy(out_sb[:, HALF:L], acc[0:1, HALF:L]).then_inc(s_act, 1)

        @block.vector
        def _(vector):
            vector.wait_ge(s_dmb, 2 * 16)   # dbc + s0h loaded
            # block 0: 512 / 512 / 1024 pieces chained via last element
            pieces = ((0, CHUNK, 1), (CHUNK, HALF, 2), (HALF, L, 3))
            for n, (lo, hi, gate) in enumerate(pieces):
                vector.wait_ge(s_act, gate)
                if n:
                    vector.wait_ge(s_dve, n)  # RAW on previous piece's tail
                init = (s0h_sb[:, 0:1] if lo == 0
                        else W[0][:, lo - 1:lo])
                vector.tensor_tensor_scan(
                    W[0][:, lo:hi], dbc_sb[:, lo:hi], G[0][:, lo:hi], init,
                    op0=add, op1=mult).then_inc(s_dve, 1)
            for b in range(1, NBLK - 1):
                if b >= NS:
                    vector.wait_ge(s_red, b - 3)  # dens(b-NS) freed W tile
                vector.wait_ge(s_act, act_end(2 * b + 1))
                vector.tensor_tensor_scan(
                    W[b % NS][:, :], dbc_sb[:, :], G[b % 2][:, :],
                    s0h_sb[:, b:b + 1],
                    op0=add, op1=mult).then_inc(s_dve, 1)
            # block 19: 1536 / 512 so the tail reduction starts early
            b = NBLK - 1
            vector.wait_ge(s_red, b - 3)
            vector.wait_ge(s_act, act_end(2 * b + 1))
            SPL = 3 * CHUNK
            vector.tensor_tensor_scan(
                W[b % NS][:, 0:SPL], dbc_sb[:, 0:SPL], G[b % 2][:, 0:SPL],
                s0h_sb[:, b:b + 1],
                op0=add, op1=mult).then_inc(s_dve, 1)
            vector.wait_ge(s_dve, NBLK + 2)
            vector.tensor_tensor_scan(
                W[b % NS][:, SPL:L], dbc_sb[:, SPL:L], G[b % 2][:, SPL:L],
                W[b % NS][:, SPL - 1:SPL],
                op0=add, op1=mult).then_inc(s_dve, 1)

    return nc


def make_core_inputs(x, mesh_points, raw_density, current_state, current_field,
                     h_min, h_range):
    """Host-side preprocessing. Returns (in_maps, h, mu, dens_sum)."""
    f = np.float32
    x = np.asarray(x, f)
    h = ((x - f(h_min)) / f(h_range)).astype(f)
    hprev = np.empty_like(h)
    hprev[0] = f(current_field)
    hprev[1:] = h[:-1]
    mu = (h > hprev).astype(f)   # rising steps
    md = (h < hprev).astype(f)   # falling steps
    me = 1.0 - mu - md           # equal steps

    bias_g = (mu * (-100.0 * h) + md * (100.0 * h) + me * BIG).astype(f)
    xg_row = np.stack([mu, md, bias_g], axis=0).astype(f)        # [3, L]

    # d_t = mu_{t-1} - mu_t with mu_0 := 0, broadcast across partitions
    d_row = np.empty(L, f)
    d_row[0] = -mu[0]
    d_row[1:] = mu[:-1] - mu[1:]
    dbc = np.broadcast_to(d_row, (P, L)).copy()

    mesh = np.asarray(mesh_points, f)
    alpha = np.full(CAP, 0.5, f)
    beta = np.full(CAP, 0.5, f)
    alpha[:M] = mesh[:, 1]
    beta[:M] = mesh[:, 0]

    raw = np.asarray(raw_density, f)
    dens_full = np.zeros(CAP, f)
    dens_full[:M] = np.logaddexp(raw, f(0.0)).astype(f)  # softplus
    dens_sum = np.sum(dens_full[:M], dtype=f)

    s0_full = np.zeros(CAP, f)
    s0_full[:M] = ((np.asarray(current_state, f) + f(1.0)) * f(0.5))

    in_maps = []
    for c in range(NCORES):
        sl = slice(c * RCORE, (c + 1) * RCORE)
        a_c, b_c = alpha[sl], beta[sl]
        wg = np.stack([100.0 * a_c, -100.0 * b_c, np.ones(RCORE, f)], 0)
        in_maps.append({
            "xgwg": np.concatenate([xg_row, wg.astype(f)], axis=1),
            # [P, NBLK]: column b = relays b*128..b*128+127 of this core
            "s0h": s0_full[sl].reshape(NBLK, P).T.copy(),
            "dbc": dbc,
            "dens": dens_full[sl].reshape(NBLK, P).T.astype(
                ml_dtypes.bfloat16),
        })
    return in_maps, h, mu, dens_sum


def kernel(x, mesh_points, raw_density, offset, scale, slope,
           current_state, current_field, h_min, h_range):
    global _last_results
    f = np.float32
    in_maps, h, mu, dens_sum = make_core_inputs(
        x, mesh_points, raw_density, current_state, current_field,
        h_min, h_range)

    nc = build_program()
    trace = os.environ.get("KERNEL_TRACE", "0") == "1"
    res = run_bass_kernel_spmd(nc, in_maps, list(range(NCORES)), trace=trace)
    _last_results = res

    num = np.zeros(L, f)
    for r in res.results:
        num += r["partial"].reshape(L)
    num += mu * dens_sum          # undo the w = shat - mu substitution
    m = (f(2.0) * num / dens_sum - f(1.0)).astype(f)

    scale = np.asarray(scale, f)
    offset = np.asarray(offset, f)
    slope = np.asarray(slope, f)
    return (scale * m + offset + h * slope).astype(f)


# revision 15
# speedup vs baseline: 1.0054x; 1.0054x over previous
"""Preisach hysteresis (nn_BaseHysteresis) Bass kernel for 8 TRN2 cores.

Math: with shat = (s+1)/2 the per-relay update is affine, shat' = g*shat + c:
    rising  (h > h_prev): g = sigmoid(100*(alpha-h)), c = 1-g
    falling (h < h_prev): g = sigmoid(100*(h-beta)),  c = 0
    equal              : g = 1, c = 0
Since c = mu*(1-g) exactly (mu = rising indicator), the substitution
    w_t = shat_t - mu_t ,  d_t = mu_{t-1} - mu_t   (mu_0 := 0)
turns the recurrence into  w_t = g_t * (w_{t-1} + d_t)  -- no c at all.
The per-step reduction Sum_p dens_p*shat_p,t = Sum_p dens_p*w_p,t
+ mu_t * Sum_p dens_p is fixed up on the host.

Per core (2560 relays = 20 blocks of 128):
 - PE builds arg_g = wg^T @ xg as f32r matmuls into PSUM (half-block
   granularity, double-buffered),
 - ScalarE applies sigmoid PSUM -> G[b%2] (f32, two 1024 halves),
 - DVE runs one tensor_tensor_scan per block (d broadcast tile + G),
   back-to-back -- the 20 scans * ~4.46us are the critical path,
 - PE reduces dens^T @ W into a [1,2048] PSUM accumulator (f32r, lag 2).
GpSimd only memsets the warmup tile: concurrent GpSimd tensor ops halve
DVE scan throughput (measured), so it must stay idle during scans.
Host sums the 8 partial reductions, adds mu*dens_sum, applies the affine.
"""

import os
from contextlib import ExitStack

import ml_dtypes
import numpy as np

import concourse.bass as bass
import concourse.mybir as mybir
from concourse.bass_utils import run_bass_kernel_spmd

F32 = mybir.dt.float32
F32R = mybir.dt.float32r
BF16 = mybir.dt.bfloat16

L = 2048            # field sequence length
P = 128             # SBUF partitions
CHUNK = 512         # PSUM bank free size (f32)
HALF = 1024
NBLK = 20           # relay blocks per core
RCORE = NBLK * P    # relays per core (2560)
NCORES = 8
CAP = RCORE * NCORES  # padded mesh size 20480
M = 20100
BIG = 10000.0
NS = 4              # W-tile ring depth (>= LAG+2, LAG=2)

_last_results = None  # BassKernelResults of the most recent run (for test.py)


def _scan_end(b):
    """s_dve value after block b's scan completes. Block 0 is split
    512/512/1024 (3 instructions), block 19 is split 1536/512 (2)."""
    if b <= 0:
        return 3
    if b >= NBLK - 1:
        return NBLK + 3
    return b + 3


def build_program() -> bass.Bass:
    nc = bass.Bass("TRN2", target_bir_lowering=False)

    xg_d = nc.dram_tensor("xg", [3, L], F32R, kind="ExternalInput")
    wg_d = nc.dram_tensor("wg", [3, RCORE], F32R, kind="ExternalInput")
    s0h_d = nc.dram_tensor("s0h", [P, NBLK], F32, kind="ExternalInput")
    dbc_d = nc.dram_tensor("dbc", [P, L], F32, kind="ExternalInput")
    dens_d = nc.dram_tensor("dens", [P, NBLK], BF16, kind="ExternalInput")
    out_d = nc.dram_tensor("partial", [1, L], F32, kind="ExternalOutput")

    sig = mybir.ActivationFunctionType.Sigmoid
    mult = mybir.AluOpType.mult
    add = mybir.AluOpType.add

    with ExitStack() as ctx:
        xg_sb = ctx.enter_context(nc.sbuf_tensor([128, L], F32R))
        wg_sb = ctx.enter_context(nc.sbuf_tensor([128, RCORE], F32R))
        s0h_sb = ctx.enter_context(nc.sbuf_tensor([P, NBLK], F32))
        dbc_sb = ctx.enter_context(nc.sbuf_tensor([P, L], F32))
        dens_sb = ctx.enter_context(nc.sbuf_tensor([P, NBLK], BF16))
        warm = ctx.enter_context(nc.sbuf_tensor([3, CHUNK], BF16))
        scratch = ctx.enter_context(nc.sbuf_tensor([1, 32], F32))
        G = [ctx.enter_context(nc.sbuf_tensor(f"g{i}", [P, L], F32))
             for i in range(2)]
        W = [ctx.enter_context(nc.sbuf_tensor(f"w{i}", [P, L], BF16))
             for i in range(NS)]
        out_sb = ctx.enter_context(nc.sbuf_tensor([1, L], F32))

        PH = [ctx.enter_context(nc.psum_tensor(f"ph{i}", [P, HALF], F32))
              for i in range(2)]
        acc = ctx.enter_context(nc.psum_tensor([1, L], F32))

        s_dma = ctx.enter_context(nc.semaphore("s_dma"))
        s_dm2 = ctx.enter_context(nc.semaphore("s_dm2"))
        s_dmb = ctx.enter_context(nc.semaphore("s_dmb"))
        s_dmc = ctx.enter_context(nc.semaphore("s_dmc"))
        s_warm = ctx.enter_context(nc.semaphore("s_warm"))
        s_arg = ctx.enter_context(nc.semaphore("s_arg"))
        s_red = ctx.enter_context(nc.semaphore("s_red"))
        s_act = ctx.enter_context(nc.semaphore("s_act"))
        s_dve = ctx.enter_context(nc.semaphore("s_dve"))
        block = ctx.enter_context(nc.Block())

        # act completion counts: block0 = 3 acts (512/512/1024),
        # halves x>=2 are act number x+1 -> count x+2; copies follow.
        def act_end(x):
            return (2, 3)[x] if x < 2 else x + 2

        @block.sync
        def _(sync):
            # xg/wg replicated at partition bases 0/32/64 so the PE's stream
            # reads don't concentrate on partitions 0-2 (which would stall
            # the partition-lockstep DVE scans). Copy 0 lands first and
            # gates blocks 0-1; dbc/s0h are issued from the scalar queue in
            # parallel to halve the serialized dma_start issue latency.
            sync.dma_start(xg_sb[0:3, :], xg_d[:, :]).then_inc(s_dma, 16)
            sync.dma_start(wg_sb[0:3, :], wg_d[:, :]).then_inc(s_dma, 16)
            for c in (1, 2):
                p0 = 32 * c
                sync.dma_start(xg_sb[p0:p0 + 3, :], xg_d[:, :]
                               ).then_inc(s_dm2, 16)
                sync.dma_start(wg_sb[p0:p0 + 3, :], wg_d[:, :]
                               ).then_inc(s_dm2, 16)
            sync.dma_start(dens_sb[:, :], dens_d[:, :]).then_inc(s_dmc, 16)
            n_acts = 3 + 2 * (NBLK - 1)
            sync.wait_ge(s_act, n_acts + 1)
            sync.dma_start(out_d[:, 0:HALF], out_sb[:, 0:HALF]
                           ).then_inc(s_dma, 16)
            sync.wait_ge(s_act, n_acts + 2)
            sync.dma_start(out_d[:, HALF:L], out_sb[:, HALF:L]
                           ).then_inc(s_dma, 16)

        @block.gpsimd
        def _(gpsimd):
            gpsimd.memset(warm[:, :], 0.0).then_inc(s_warm, 1)

        @block.tensor
        def _(tensor):
            # brief p-state warmup while the DMAs land
            tensor.wait_ge(s_warm, 1)
            for _ in range(2):
                tensor.matmul(PH[0][:, 0:CHUNK], warm[:, 0:P], warm[:, :],
                              start=True, stop=True, skip_group_check=True)
            tensor.wait_ge(s_dma, 2 * 16)   # xg, wg copy 0 loaded
            dens_gate = False

            def emit_dens(j):
                nonlocal dens_gate
                if not dens_gate:
                    tensor.wait_ge(s_dmc, 16)
                    dens_gate = True
                dj = dens_sb[:, j:j + 1]
                wj = W[j % NS]
                if j == NBLK - 1:
                    gates = [NBLK + 2, None, None, NBLK + 3]
                else:
                    gates = [_scan_end(j), None, None, None]
                for k in range(4):
                    sl = slice(k * CHUNK, (k + 1) * CHUNK)
                    if gates[k] is not None:
                        tensor.wait_ge(s_dve, gates[k])
                    mm = tensor.matmul(acc[0:1, sl], dj, wj[:, sl],
                                       start=(j == 0), stop=(j == NBLK - 1),
                                       skip_group_check=True)
                    if k == 3:
                        mm.then_inc(s_red, 1)
                    elif k == 1 and j == NBLK - 1:
                        mm.then_inc(s_red, 1)

            for b in range(NBLK):
                if b == 2:
                    tensor.wait_ge(s_dm2, 4 * 16)   # replicas 1,2 loaded
                for h in range(2):
                    x = 2 * b + h
                    if x >= 2:
                        tensor.wait_ge(s_act, act_end(x - 2))  # PH free
                    lo = h * HALF
                    for j in range(2):
                        # blocks 0-1 only have replica 0 available yet
                        p0 = 0 if b < 2 else 32 * ((2 * h + j) % 3)
                        tensor.matmul(
                            PH[x % 2][:, j * CHUNK:(j + 1) * CHUNK],
                            wg_sb[p0:p0 + 3, b * P:(b + 1) * P],
                            xg_sb[p0:p0 + 3,
                                  lo + j * CHUNK:lo + (j + 1) * CHUNK],
                            start=True, stop=True, skip_group_check=True
                        ).then_inc(s_arg, 1)
                if b >= 2:
                    emit_dens(b - 2)
            emit_dens(NBLK - 2)
            emit_dens(NBLK - 1)

        @block.scalar
        def _(scalar):
            # dbc/s0h loads issue here, in parallel with the sync queue's
            scalar.dma_start(dbc_sb[:, :], dbc_d[:, :]).then_inc(s_dmb, 16)
            scalar.dma_start(s0h_sb[:, :], s0h_d[:, :]).then_inc(s_dmb, 16)
            # sigmoid act-table preload off the critical path
            scalar.wait_ge(s_warm, 1)
            scalar.activation(scratch[:, :], warm[0:1, 0:32], sig)
            # block 0: two 512-wide acts as soon as each arg matmul lands,
            # then one 1024 act for the second half
            for q in range(2):
                scalar.wait_ge(s_arg, q + 1)
                qs = slice(q * CHUNK, (q + 1) * CHUNK)
                scalar.activation(G[0][:, qs], PH[0][:, qs], sig
                                  ).then_inc(s_act, 1)
            scalar.wait_ge(s_arg, 4)
            scalar.activation(G[0][:, HALF:L], PH[1][:, :], sig
                              ).then_inc(s_act, 1)
            for b in range(1, NBLK):
                for h in range(2):
                    x = 2 * b + h
                    scalar.wait_ge(s_arg, 2 * x + 2)
                    if b >= 2:
                        scalar.wait_ge(s_dve, _scan_end(b - 2))  # G free
                    hsl = slice(h * HALF, (h + 1) * HALF)
                    scalar.activation(G[b % 2][:, hsl], PH[x % 2][:, :], sig
                                      ).then_inc(s_act, 1)
            scalar.wait_ge(s_red, NBLK)      # dens(19) chunks 0-1 done
            scalar.copy(out_sb[:, 0:HALF], acc[0:1, 0:HALF]).then_inc(s_act, 1)
            scalar.wait_ge(s_red, NBLK + 1)  # dens(19) chunk 3 done
            scalar.copy(out_sb[:, HALF:L], acc[0:1, HALF:L]).then_inc(s_act, 1)

        @block.vector
        def _(vector):
            vector.wait_ge(s_dmb, 2 * 16)   # dbc + s0h loaded
            # block 0: 512 / 512 / 1024 pieces chained via last element
            pieces = ((0, CHUNK, 1), (CHUNK, HALF, 2), (HALF, L, 3))
            for n, (lo, hi, gate) in enumerate(pieces):
                vector.wait_ge(s_act, gate)
                if n:
                    vector.wait_ge(s_dve, n)  # RAW on previous piece's tail
                init = (s0h_sb[:, 0:1] if lo == 0
                        else W[0][:, lo - 1:lo])
                vector.tensor_tensor_scan(
                    W[0][:, lo:hi], dbc_sb[:, lo:hi], G[0][:, lo:hi], init,
                    op0=add, op1=mult).then_inc(s_dve, 1)
            for b in range(1, NBLK - 1):
                if b >= NS:
                    vector.wait_ge(s_red, b - 3)  # dens(b-NS) freed W tile
                vector.wait_ge(s_act, act_end(2 * b + 1))
                vector.tensor_tensor_scan(
                    W[b % NS][:, :], dbc_sb[:, :], G[b % 2][:, :],
                    s0h_sb[:, b:b + 1],
                    op0=add, op1=mult).then_inc(s_dve, 1)
            # block 19: 1536 / 512 so the tail reduction starts early
            b = NBLK - 1
            vector.wait_ge(s_red, b - 3)
            vector.wait_ge(s_act, act_end(2 * b + 1))
            SPL = 3 * CHUNK
            vector.tensor_tensor_scan(
                W[b % NS][:, 0:SPL], dbc_sb[:, 0:SPL], G[b % 2][:, 0:SPL],
                s0h_sb[:, b:b + 1],
                op0=add, op1=mult).then_inc(s_dve, 1)
            vector.wait_ge(s_dve, NBLK + 2)
            vector.tensor_tensor_scan(
                W[b % NS][:, SPL:L], dbc_sb[:, SPL:L], G[b % 2][:, SPL:L],
                W[b % NS][:, SPL - 1:SPL],
                op0=add, op1=mult).then_inc(s_dve, 1)

    return nc


def make_core_inputs(x, mesh_points, raw_density, current_state, current_field,
                     h_min, h_range):
    """Host-side preprocessing. Returns (in_maps, h, mu, dens_sum)."""
    f = np.float32
    x = np.asarray(x, f)
    h = ((x - f(h_min)) / f(h_range)).astype(f)
    hprev = np.empty_like(h)
    hprev[0] = f(current_field)
    hprev[1:] = h[:-1]
    mu = (h > hprev).astype(f)   # rising steps
    md = (h < hprev).astype(f)   # falling steps
    me = 1.0 - mu - md           # equal steps

    bias_g = (mu * (-100.0 * h) + md * (100.0 * h) + me * BIG).astype(f)
    xg_row = np.stack([mu, md, bias_g], axis=0).astype(f)        # [3, L]

    # d_t = mu_{t-1} - mu_t with mu_0 := 0, broadcast across partitions
    d_row = np.empty(L, f)
    d_row[0] = -mu[0]
    d_row[1:] = mu[:-1] - mu[1:]
    dbc = np.broadcast_to(d_row, (P, L)).copy()

    mesh = np.asarray(mesh_points, f)
    alpha = np.full(CAP, 0.5, f)
    beta = np.full(CAP, 0.5, f)
    alpha[:M] = mesh[:, 1]
    beta[:M] = mesh[:, 0]

    raw = np.asarray(raw_density, f)
    dens_full = np.zeros(CAP, f)
    dens_full[:M] = np.logaddexp(raw, f(0.0)).astype(f)  # softplus
    dens_sum = np.sum(dens_full[:M], dtype=f)

    s0_full = np.zeros(CAP, f)
    s0_full[:M] = ((np.asarray(current_state, f) + f(1.0)) * f(0.5))

    in_maps = []
    for c in range(NCORES):
        sl = slice(c * RCORE, (c + 1) * RCORE)
        a_c, b_c = alpha[sl], beta[sl]
        wg = np.stack([100.0 * a_c, -100.0 * b_c, np.ones(RCORE, f)], 0)
        in_maps.append({
            "xg": xg_row,
            "wg": wg.astype(f),
            # [P, NBLK]: column b = relays b*128..b*128+127 of this core
            "s0h": s0_full[sl].reshape(NBLK, P).T.copy(),
            "dbc": dbc,
            "dens": dens_full[sl].reshape(NBLK, P).T.astype(
                ml_dtypes.bfloat16),
        })
    return in_maps, h, mu, dens_sum


def kernel(x, mesh_points, raw_density, offset, scale, slope,
           current_state, current_field, h_min, h_range):
    global _last_results
    f = np.float32
    in_maps, h, mu, dens_sum = make_core_inputs(
        x, mesh_points, raw_density, current_state, current_field,
        h_min, h_range)

    nc = build_program()
    trace = os.environ.get("KERNEL_TRACE", "0") == "1"
    res = run_bass_kernel_spmd(nc, in_maps, list(range(NCORES)), trace=trace)
    _last_results = res

    num = np.zeros(L, f)
    for r in res.results:
        num += r["partial"].reshape(L)
    num += mu * dens_sum          # undo the w = shat - mu substitution
    m = (f(2.0) * num / dens_sum - f(1.0)).astype(f)

    scale = np.asarray(scale, f)
    offset = np.asarray(offset, f)
    slope = np.asarray(slope, f)
    return (scale * m + offset + h * slope).astype(f)
